# revision 5
# baseline (speedup 1.0000x reference)
"""Trainium2 Bass kernel for BiAttention (b=8, n=m=1024, d=512).

Sharding: data-parallel over batch — one batch element per NeuronCore,
8 cores, no cross-core communication.

Per-core algorithm (softmax shift-invariance folds the Linear(3d,1)
row/col terms, bias, and both padding masks into per-row/col exponent
weights g1 = exp(s1+logm1), g2 = exp(s2+logm2); logits ~ N(0,1) so raw
exp is safe):

  sim      = (x1*w3) @ x2^T              (n, m)   [tri term only]
  E        = exp(sim)                    (bf16 + fp8 copies)
  ET       = E^T                         via DMA xbar transpose (bf16->fp8)
  U_row    = ET8^T @ (x2*g2/4)  -> c2q = U_row/den1,  den1 = g2c8 @ ET8
  U_col    = E8^T  @ (x1*g1)    -> q2c = U_col/den2,  den2 = g1c8 @ E8
  V        = ET8^T @ Q2C        -> q2c_att = V * rden1/SQ
  out      = [x1, c2q, x1*c2q, x1*q2c_att]        (n, 4d)

Precision plan (rel-err budget ~1%, gate 2e-2): sim matmul in bf16 with
x1w3T/x2T built by DMA xbar transposes (offloads both the PE transposes
and their PSUM evictions); the three big weighted-sum matmuls run in
fp8e4 with DoubleRow perf mode (two contraction tiles per instruction);
all sums accumulate in f32 PSUM and the softmax divisions are exact f32.

Engines: PE does only matmuls; PSUM evictions split Act/DVE; SBUF->SBUF
conversions on GpSimd (which cannot touch PSUM); x1/x2 xbar transposes
ride the Act HWDGE ring while loads, E transposes, and all stores ride
the SP ring. A few identity transposes at the head keep the PE HAM clock
from throttling to 1.2 GHz during the load phase.

Mask-suffix specialization: 128-row tiles fully masked at the end of
either sequence are skipped in the contractions (host inspects masks and
dispatches to a NEFF compiled for that (kn, km)); partially-masked tiles
are exact via the exponent weights.
"""

import numpy as np
from contextlib import ExitStack

import concourse.bacc as bacc
import concourse.tile as tile
import concourse.mybir as mybir
from concourse.bass_utils import run_bass_kernel_spmd
from concourse.masks import make_identity

F32 = mybir.dt.float32
BF16 = mybir.dt.bfloat16
F8 = mybir.dt.float8e4
U8 = mybir.dt.uint8
EXP = mybir.ActivationFunctionType.Exp
COPY = mybir.ActivationFunctionType.Copy
DR = mybir.MatmulPerfMode.DoubleRow

P = 128
N = 1024          # x1 rows
M = 1024          # x2 rows
D = 512           # feature dim
NT, MT, DC = N // P, M // P, D // P
NEGB = -30000.0   # exp(x + NEGB) == 0.0 exactly for |x| < 80
SX = 32.0         # x1w3 prescale (keeps bf16 products well-scaled)
SQ = 16.0         # q2c prescale (keeps fp8 Q2C out of denormals)
LN4 = 1.3862943611198906

N_CORES = 8

_CACHE = {}


def _chunks(width, step=512):
    out = []
    o = 0
    while o < width:
        w = min(step, width - o)
        out.append((o, w))
        o += w
    return out


def _pairs(k):
    """(k0, is_pair) covering range(k) with DoubleRow pairs + odd tail."""
    out = [(2 * i, True) for i in range(k // 2)]
    if k % 2:
        out.append((k - 1, False))
    return out


def _build(kn, km):
    vm = km * P
    nc = bacc.Bacc("TRN2", target_bir_lowering=False, debug=False)
    x1d = nc.dram_tensor("x1", [N, D], F32, kind="ExternalInput").ap()
    x2d = nc.dram_tensor("x2", [M, D], F32, kind="ExternalInput").ap()
    m1d = nc.dram_tensor("x1_mask", [N], U8, kind="ExternalInput").ap()
    m2d = nc.dram_tensor("x2_mask", [M], U8, kind="ExternalInput").ap()
    wd = nc.dram_tensor("W", [3 * D], F32, kind="ExternalInput").ap()
    outd = nc.dram_tensor("out", [N, 4 * D], F32, kind="ExternalOutput").ap()

    x1r_d = x1d.rearrange("(t p) d -> p t d", p=P)
    x2r_d = x2d.rearrange("(t p) d -> p t d", p=P)
    out_r = outd.rearrange("(t p) e -> p t e", p=P)

    with tile.TileContext(nc) as tc, ExitStack() as ctx:
        big = ctx.enter_context(tc.tile_pool(name="big", bufs=1))
        rows = ctx.enter_context(tc.tile_pool(name="rows", bufs=1))
        work = ctx.enter_context(tc.tile_pool(name="work", bufs=3))
        psb = ctx.enter_context(tc.tile_pool(name="psb", bufs=2, space="PSUM"))
        psu = ctx.enter_context(tc.tile_pool(name="psu", bufs=2, space="PSUM"))
        psd = ctx.enter_context(tc.tile_pool(name="psd", bufs=2, space="PSUM"))

        # ---------------- constants ----------------
        ident = big.tile([P, P], F32)
        make_identity(nc, ident)
        identb = big.tile([P, P], BF16)
        nc.vector.tensor_copy(identb[:], ident[:])
        onesrow = rows.tile([1, P], F32)
        nc.vector.memset(onesrow[:], 1.0)
        negln4 = big.tile([P, 1], F32)
        nc.vector.memset(negln4[:], -LN4)

        # ---------------- DMA loads (SP ring) ----------------
        wrow = rows.tile([1, 12 * P], F32)
        nc.sync.dma_start(wrow[:], wd.rearrange("(a n) -> a n", a=1))
        m1row = rows.tile([1, N], U8)
        nc.sync.dma_start(m1row[:], m1d.rearrange("(a n) -> a n", a=1))
        m2row = rows.tile([1, M], U8)
        nc.sync.dma_start(m2row[:], m2d.rearrange("(a n) -> a n", a=1))

        x1n = big.tile([P, NT, D], F32)
        x2n = big.tile([P, km, D], F32)
        nc.sync.dma_start(x2n[:], x2r_d[:, 0:km, :])
        for t0 in range(0, NT, 2):
            nc.sync.dma_start(x1n[:, t0:t0 + 2, :], x1r_d[:, t0:t0 + 2, :])

        # ---------------- PE warmup (keeps the HAM clock busy) -------------
        def warm(n_, base):
            for i in range(n_):
                pw = psd.tile([P, P], BF16, tag="small", name=f"warm{base + i}")
                nc.tensor.transpose(pw[:], identb[:], identb[:])

        warm(10, 0)

        # ---------------- W prep ----------------
        pwc = psd.tile([P, 16], F32, tag="small", name="pwc")
        for c in range(12):
            nc.tensor.transpose(pwc[:, c:c + 1], wrow[0:1, c * P:(c + 1) * P],
                                ident[0:1, 0:1])
        wcols = big.tile([P, 12], F32)
        nc.vector.tensor_copy(wcols[:], pwc[:, 0:12])
        w3rec = big.tile([P, 4], F32)
        nc.vector.reciprocal(w3rec[:], wcols[:, 8:12])
        u1f = big.tile([P, 4], F32)
        nc.vector.tensor_mul(u1f[:], wcols[:, 0:4], w3rec[:])
        u1r = big.tile([P, 4], BF16)       # w1/(w3*SX): recovers s1 from x1w3T
        nc.vector.tensor_scalar_mul(u1r[:], u1f[:], 1.0 / SX)
        w2r = big.tile([P, 4], BF16)
        nc.vector.tensor_copy(w2r[:], wcols[:, 4:8])
        # W3bc = broadcast row of w3*SX (outer product), for x1s = x1 * w3 * SX
        psW = psd.tile([P, D], F32, tag="small", name="psW")
        nc.tensor.matmul(psW[:], onesrow[0:1, :], wrow[0:1, 2 * D:3 * D],
                         start=True, stop=True)
        W3bc = big.tile([P, D], F32)
        nc.vector.tensor_scalar_mul(W3bc[:], psW[:], SX)

        logm1 = rows.tile([1, N], F32)
        nc.vector.tensor_scalar_mul(logm1[:], m1row[:], NEGB)
        logm2 = rows.tile([1, M], F32)
        nc.vector.tensor_scalar_mul(logm2[:], m2row[:], NEGB)

        warm(10, 10)

        # ---------------- conversions + xbar transposes ----------------
        x2b = big.tile([P, km, D], BF16)
        x1s = big.tile([P, NT, D], BF16)
        x1w3T = big.tile([P, NT, DC, P], BF16)   # (d_lo, t, c, n_lo)
        x2T = big.tile([P, km, DC, P], BF16)     # (d_lo, u, c, m_lo)

        # gpsimd: x2 bf16 then x1*w3 bf16, in load-arrival order
        for u0, uw in _chunks(km, 2):
            nc.gpsimd.tensor_copy(x2b[:, u0:u0 + uw, :], x2n[:, u0:u0 + uw, :])
        for t in range(NT):
            nc.gpsimd.tensor_mul(x1s[:, t, :], x1n[:, t, :], W3bc[:])

        # x1/x2 transposes ride the Act HWDGE ring (SP ring is busy loading);
        # later x1 triggers are interleaved into the sim loop below so they
        # don't head-of-line-block the E evictions.
        nc.scalar.dma_start_transpose(x2T[:], x2b[:])
        nc.scalar.dma_start_transpose(x1w3T[:, 0:2, :, :], x1s[:, 0:2, :])
        nc.scalar.dma_start_transpose(x1w3T[:, 2:4, :, :], x1s[:, 2:4, :])

        # ---------------- sim + s1/s2 + E ----------------
        E_raw = big.tile([P, NT, vm], BF16)      # exp(sim), n-major
        E8 = big.tile([P, kn, vm], F8)           # fp8 copy (U_col lhsT)
        ETraw = big.tile([P, NT, km, P], BF16)   # (m_lo, t, u, n_lo)
        ET8 = big.tile([P, NT, km, P], F8)
        x1aug = big.tile([P, kn, D], F8)         # x1 * g1
        x2aug = big.tile([P, km, D], F8)         # x2 * g2/4
        mch = _chunks(vm)

        def sim_tile(t):
            ps = psb.tile([P, 1024], F32, tag="ps_sim", name=f"sim{t}")
            for off, w in mch:
                u0, nu = off // P, w // P
                for c in range(DC):
                    nc.tensor.matmul(ps[:, off:off + w],
                                     x1w3T[:, t, c, :],
                                     x2T[:, u0:u0 + nu, c, :],
                                     start=(c == 0), stop=(c == DC - 1))
            # one wide eviction (Act); exp undoes the SX prescale
            nc.scalar.activation(E_raw[:, t, :], ps[:, 0:vm], EXP, scale=1.0 / SX)
            # E transposes ride the SP ring (loads are done by the time they go)
            nc.sync.dma_start_transpose(ETraw[:, t, :, :], E_raw[:, t, :])
            # fp8 copy for the U_col contraction
            if t < kn:
                nc.gpsimd.tensor_copy(E8[:, t, :], E_raw[:, t, :])

        def s_chunk(name, lhs, rhsT, brow, logm, off, w):
            t0, ntile = off // P, w // P
            ps_s = psd.tile([1, D], F32, tag="small", name=f"ps{name}{off}")
            for c in range(DC):
                nc.tensor.matmul(ps_s[0:1, 0:w], lhs[:, c:c + 1],
                                 rhsT[:, t0:t0 + ntile, c, :],
                                 start=(c == 0), stop=(c == DC - 1))
            nc.vector.tensor_add(brow[:, off:off + w], ps_s[0:1, 0:w],
                                 logm[:, off:off + w])

        def col_of(name, brow, nt):
            pbc = psd.tile([P, 16], F32, tag="small", name=f"pbc{name}")
            for t in range(nt):
                nc.tensor.transpose(pbc[:, t:t + 1], brow[0:1, t * P:(t + 1) * P],
                                    ident[0:1, 0:1])
            return pbc

        b1row = rows.tile([1, N], F32)
        b2row = rows.tile([1, M], F32)

        # s2 first (x2T is complete before any x1w3T pair), then s1 chunk A
        for off, w in _chunks(vm):
            s_chunk("b2", w2r, x2T, b2row, logm2, off, w)
        pbc2 = col_of("b2", b2row, km)
        g2c4 = big.tile([P, km], F32)
        nc.scalar.activation(g2c4[:], pbc2[:, 0:km], EXP, bias=negln4[:, 0:1])
        g2c8 = big.tile([P, 8, 16], F8)
        for u in range(km):
            nc.vector.tensor_copy(g2c8[:, u, 0:1], g2c4[:, u:u + 1])
        s_chunk("b1", u1r, x1w3T, b1row, logm1, 0, 512)

        # x2aug on Act, interleaved with E evictions (ready long before U_row)
        sim_tile(0)
        for u in range(0, km, 2):
            uw = min(2, km - u)
            for uu in range(u, u + uw):
                nc.scalar.activation(x2aug[:, uu, :], x2n[:, uu, :], COPY,
                                     scale=g2c4[:, uu:uu + 1])
            if u == 0:
                nc.scalar.dma_start_transpose(x1w3T[:, 4:6, :, :], x1s[:, 4:6, :])
        sim_tile(1)
        sim_tile(2)
        nc.scalar.dma_start_transpose(x1w3T[:, 6:8, :, :], x1s[:, 6:8, :])
        sim_tile(3)
        s_chunk("b1", u1r, x1w3T, b1row, logm1, 512, 512)
        pbc1 = col_of("b1", b1row, NT)
        g1c = big.tile([P, NT], F32)
        nc.scalar.activation(g1c[:], pbc1[:, 0:NT], EXP)
        g1c8 = big.tile([P, 8, 16], F8)
        for k in range(kn):
            nc.vector.tensor_copy(g1c8[:, k, 0:1], g1c[:, k:k + 1])
        # x1aug on DVE right after g1c — U_col is gated on it
        for t in range(kn):
            nc.vector.tensor_scalar_mul(x1aug[:, t, :], x1n[:, t, :],
                                        g1c[:, t:t + 1])
        for t in range(4, NT):
            sim_tile(t)

        # ET8 = fp8(ETraw): gpsimd takes the first tiles, DVE the tail
        for t in range(6):
            nc.gpsimd.tensor_copy(ET8[:, t, :, :], ETraw[:, t, :, :])
        for t in range(6, NT):
            nc.vector.tensor_copy(ET8[:, t, :, :], ETraw[:, t, :, :])

        # block 0 = x1 (SP ring, queued after the E transposes)
        nc.sync.dma_start(out_r[:, 0:4, 0:D], x1n[:, 0:4, :])
        nc.sync.dma_start(out_r[:, 4:8, 0:D], x1n[:, 4:8, :])

        # ---------------- den2, U_col -> Q2C ----------------
        kp_n, kp_m = _pairs(kn), _pairs(km)
        den2row = rows.tile([1, vm], F32)
        for off, w in _chunks(vm):
            ps_d = psd.tile([1, D], F32, tag="small", name=f"psden2{off}")
            for i, (k0, pair) in enumerate(kp_n):
                last = i == len(kp_n) - 1
                if pair:
                    nc.tensor.matmul(ps_d[0:1, 0:w], g1c8[:, k0:k0 + 2, 0:1],
                                     E8[:, k0:k0 + 2, off:off + w],
                                     start=(i == 0), stop=last, perf_mode=DR)
                else:
                    nc.tensor.matmul(ps_d[0:1, 0:w], g1c8[:, k0, 0:1],
                                     E8[:, k0, off:off + w],
                                     start=(i == 0), stop=last)
            nc.vector.tensor_copy(den2row[:, off:off + w], ps_d[0:1, 0:w])
        pdc2 = col_of("d2", den2row, km)
        rden2 = big.tile([P, km], F32)
        nc.vector.reciprocal(rden2[:], pdc2[:, 0:km])
        rQ = big.tile([P, km], F32)          # rden2 * g2/4 * SQ
        nc.vector.tensor_mul(rQ[:], rden2[:], g2c4[:])
        nc.vector.tensor_scalar_mul(rQ[:], rQ[:], SQ)

        Q2C = big.tile([P, km, D], F8)       # q2c * g2/4 * SQ
        for u in range(km):
            pu = psu.tile([P, D], F32, tag="ps_uv", name=f"pu{u}")
            for i, (k0, pair) in enumerate(kp_n):
                last = i == len(kp_n) - 1
                if pair:
                    nc.tensor.matmul(pu[:], E8[:, k0:k0 + 2, u * P:(u + 1) * P],
                                     x1aug[:, k0:k0 + 2, :],
                                     start=(i == 0), stop=last, perf_mode=DR)
                else:
                    nc.tensor.matmul(pu[:], E8[:, k0, u * P:(u + 1) * P],
                                     x1aug[:, k0, :], start=(i == 0), stop=last)
            nc.scalar.activation(Q2C[:, u, :], pu[:], COPY, scale=rQ[:, u:u + 1])

        # ---------------- den1 ----------------
        den1row = rows.tile([1, N], F32)
        for t in range(NT):
            ps_d = psd.tile([1, D], F32, tag="small", name=f"psden1{t}")
            for i, (k0, pair) in enumerate(kp_m):
                last = i == len(kp_m) - 1
                if pair:
                    nc.tensor.matmul(ps_d[0:1, 0:P], g2c8[:, k0:k0 + 2, 0:1],
                                     ET8[:, t, k0:k0 + 2, :],
                                     start=(i == 0), stop=last, perf_mode=DR)
                else:
                    nc.tensor.matmul(ps_d[0:1, 0:P], g2c8[:, k0, 0:1],
                                     ET8[:, t, k0, :], start=(i == 0), stop=last)
            nc.vector.tensor_copy(den1row[:, t * P:(t + 1) * P], ps_d[0:1, 0:P])
        pdc1 = col_of("d1", den1row, NT)
        rden1 = big.tile([P, NT], F32)
        nc.vector.reciprocal(rden1[:], pdc1[:, 0:NT])
        rden1q = big.tile([P, NT], F32)
        nc.vector.tensor_scalar_mul(rden1q[:], rden1[:], 1.0 / SQ)

        # ---------------- U_row -> c2q ; out blocks 1, 2 ----------------
        def uv_mm(ps_ap, t, rhs):
            for i, (k0, pair) in enumerate(kp_m):
                last = i == len(kp_m) - 1
                if pair:
                    nc.tensor.matmul(ps_ap, ET8[:, t, k0:k0 + 2, :],
                                     rhs[:, k0:k0 + 2, :],
                                     start=(i == 0), stop=last, perf_mode=DR)
                else:
                    nc.tensor.matmul(ps_ap, ET8[:, t, k0, :], rhs[:, k0, :],
                                     start=(i == 0), stop=last)

        for t in range(NT):
            pr = psu.tile([P, D], F32, tag="ps_uv", name=f"pr{t}")
            uv_mm(pr[:], t, x2aug)
            combo = work.tile([P, 2 * D], F32, tag="ev", name=f"cb{t}")
            nc.scalar.activation(combo[:, 0:D], pr[:], COPY,
                                 scale=rden1[:, t:t + 1])
            nc.gpsimd.tensor_mul(combo[:, D:2 * D], x1n[:, t, :], combo[:, 0:D])
            nc.sync.dma_start(out_r[:, t, D:3 * D], combo[:])

        # ---------------- V -> q2c_att ; out block 3 ----------------
        for t in range(NT):
            pv = psu.tile([P, D], F32, tag="ps_uv", name=f"pv{t}")
            uv_mm(pv[:], t, Q2C)
            vs = work.tile([P, D], F32, tag="vs", name=f"vs{t}")
            nc.vector.tensor_scalar_mul(vs[:], pv[:], rden1q[:, t:t + 1])
            prod = work.tile([P, D], F32, tag="pd", name=f"pd{t}")
            nc.gpsimd.tensor_mul(prod[:], x1n[:, t, :], vs[:])
            nc.sync.dma_start(out_r[:, t, 3 * D:4 * D], prod[:])

    nc.compile()
    return nc


def _kept_tiles(mask):
    """Tiles (of 128) up to and including the last one with any valid row."""
    valid = ~mask.astype(bool)
    any_valid = valid.reshape(valid.shape[0], -1, P).any(axis=2).any(axis=0)
    nz = np.nonzero(any_valid)[0]
    return int(nz[-1]) + 1 if len(nz) else 1


def _get_nc(kn, km):
    key = (kn, km)
    if key not in _CACHE:
        _CACHE[key] = _build(kn, km)
    return _CACHE[key]


def _run(inputs, trace=False, trace_cores=None):
    x1 = np.ascontiguousarray(np.asarray(inputs["x1"], dtype=np.float32))
    x2 = np.ascontiguousarray(np.asarray(inputs["x2"], dtype=np.float32))
    m1 = np.ascontiguousarray(np.asarray(inputs["x1_mask"]).astype(np.uint8))
    m2 = np.ascontiguousarray(np.asarray(inputs["x2_mask"]).astype(np.uint8))
    W = np.ascontiguousarray(np.asarray(inputs["W"], dtype=np.float32))
    nc = _get_nc(_kept_tiles(m1), _kept_tiles(m2))
    in_maps = [
        {"x1": x1[i], "x2": x2[i], "x1_mask": m1[i], "x2_mask": m2[i], "W": W}
        for i in range(N_CORES)
    ]
    res = run_bass_kernel_spmd(nc, in_maps, core_ids=list(range(N_CORES)),
                               trace=trace, trace_cores=trace_cores)
    out = np.stack([res.results[i]["out"] for i in range(N_CORES)], axis=0)
    return out.astype(np.float32), res


def kernel(x1, x1_mask, x2, x2_mask, W, bias=None, **_kw):
    # bias is mathematically irrelevant: a global additive constant cancels in
    # both softmaxes, and every output term is softmax-weighted.
    out, _ = _run({"x1": x1, "x1_mask": x1_mask, "x2": x2, "x2_mask": x2_mask,
                   "W": W})
    return out


# revision 6
# speedup vs baseline: 1.3356x; 1.3356x over previous
"""Trainium2 Bass kernel for BiAttention (b=8, n=m=1024, d=512).

Sharding: data-parallel over batch — one batch element per NeuronCore,
8 cores, no cross-core communication.

Per-core algorithm (softmax shift-invariance folds the Linear(3d,1)
row/col terms, bias, and both padding masks into per-row/col exponent
weights g1 = exp(s1+logm1), g2 = exp(s2+logm2); logits ~ N(0,1) so raw
exp is safe):

  sim      = (x1*w3) @ x2^T              (n, m)   [tri term only]
  E        = exp(sim)                    (bf16 + fp8 copies)
  ET       = E^T                         via DMA xbar transpose (bf16->fp8)
  U_row    = ET8^T @ (x2*g2/4)  -> c2q = U_row/den1,  den1 = g2c8 @ ET8
  U_col    = E8^T  @ (x1*g1)    -> q2c = U_col/den2,  den2 = g1c8 @ E8
  V        = ET8^T @ Q2C        -> q2c_att = V * rden1/SQ
  out      = [x1, c2q, x1*c2q, x1*q2c_att]        (n, 4d)

Precision plan (rel-err budget ~1%, gate 2e-2): sim matmul in bf16 with
x1w3T/x2T built by DMA xbar transposes (offloads both the PE transposes
and their PSUM evictions); the three big weighted-sum matmuls run in
fp8e4 with DoubleRow perf mode (two contraction tiles per instruction);
all sums accumulate in f32 PSUM and the softmax divisions are exact f32.

Engines: PE does only matmuls; PSUM evictions split Act/DVE; SBUF->SBUF
conversions on GpSimd (which cannot touch PSUM); x1/x2 xbar transposes
ride the Act HWDGE ring while loads, E transposes, and all stores ride
the SP ring. A few identity transposes at the head keep the PE HAM clock
from throttling to 1.2 GHz during the load phase.

Mask-suffix specialization: 128-row tiles fully masked at the end of
either sequence are skipped in the contractions (host inspects masks and
dispatches to a NEFF compiled for that (kn, km)); partially-masked tiles
are exact via the exponent weights.
"""

import numpy as np
from contextlib import ExitStack

import concourse.bacc as bacc
import concourse.tile as tile
import concourse.mybir as mybir
from concourse.bass_utils import run_bass_kernel_spmd
from concourse.masks import make_identity

F32 = mybir.dt.float32
BF16 = mybir.dt.bfloat16
F8 = mybir.dt.float8e4
U8 = mybir.dt.uint8
EXP = mybir.ActivationFunctionType.Exp
COPY = mybir.ActivationFunctionType.Copy
DR = mybir.MatmulPerfMode.DoubleRow

P = 128
N = 1024          # x1 rows
M = 1024          # x2 rows
D = 512           # feature dim
NT, MT, DC = N // P, M // P, D // P
NEGB = -30000.0   # exp(x + NEGB) == 0.0 exactly for |x| < 80
SX = 32.0         # x1w3 prescale (keeps bf16 products well-scaled)
SQ = 16.0         # q2c prescale (keeps fp8 Q2C out of denormals)
LN4 = 1.3862943611198906

N_CORES = 8

_CACHE = {}


def _chunks(width, step=512):
    out = []
    o = 0
    while o < width:
        w = min(step, width - o)
        out.append((o, w))
        o += w
    return out


def _pairs(k):
    """(k0, is_pair) covering range(k) with DoubleRow pairs + odd tail."""
    out = [(2 * i, True) for i in range(k // 2)]
    if k % 2:
        out.append((k - 1, False))
    return out


def _build(kn, km):
    vm = km * P
    nc = bacc.Bacc("TRN2", target_bir_lowering=False, debug=False)
    x1d = nc.dram_tensor("x1", [N, D], F32, kind="ExternalInput").ap()
    x2d = nc.dram_tensor("x2", [M, D], F32, kind="ExternalInput").ap()
    m1d = nc.dram_tensor("x1_mask", [N], U8, kind="ExternalInput").ap()
    m2d = nc.dram_tensor("x2_mask", [M], U8, kind="ExternalInput").ap()
    wd = nc.dram_tensor("W", [3 * D], F32, kind="ExternalInput").ap()
    outd = nc.dram_tensor("out", [N, 4 * D], F32, kind="ExternalOutput").ap()

    x1r_d = x1d.rearrange("(t p) d -> p t d", p=P)
    x2r_d = x2d.rearrange("(t p) d -> p t d", p=P)
    out_r = outd.rearrange("(t p) e -> p t e", p=P)

    with tile.TileContext(nc) as tc, ExitStack() as ctx:
        big = ctx.enter_context(tc.tile_pool(name="big", bufs=1))
        rows = ctx.enter_context(tc.tile_pool(name="rows", bufs=1))
        work = ctx.enter_context(tc.tile_pool(name="work", bufs=3))
        psb = ctx.enter_context(tc.tile_pool(name="psb", bufs=2, space="PSUM"))
        psu = ctx.enter_context(tc.tile_pool(name="psu", bufs=2, space="PSUM"))
        psd = ctx.enter_context(tc.tile_pool(name="psd", bufs=2, space="PSUM"))

        # ---------------- constants ----------------
        ident = big.tile([P, P], F32)
        make_identity(nc, ident)
        identb = big.tile([P, P], BF16)
        nc.vector.tensor_copy(identb[:], ident[:])
        onesrow = rows.tile([1, P], F32)
        nc.vector.memset(onesrow[:], 1.0)
        negln4 = big.tile([P, 1], F32)
        nc.vector.memset(negln4[:], -LN4)

        # ---------------- DMA loads (SP ring) ----------------
        wrow = rows.tile([1, 12 * P], F32)
        nc.sync.dma_start(wrow[:], wd.rearrange("(a n) -> a n", a=1))
        m1row = rows.tile([1, N], U8)
        nc.sync.dma_start(m1row[:], m1d.rearrange("(a n) -> a n", a=1))
        m2row = rows.tile([1, M], U8)
        nc.sync.dma_start(m2row[:], m2d.rearrange("(a n) -> a n", a=1))

        x1n = big.tile([P, NT, D], F32)
        x2n = big.tile([P, km, D], F32)
        nc.sync.dma_start(x2n[:, 0:min(4, km), :], x2r_d[:, 0:min(4, km), :])
        nc.sync.dma_start(x1n[:, 0:2, :], x1r_d[:, 0:2, :])
        nc.sync.dma_start(x1n[:, 2:4, :], x1r_d[:, 2:4, :])
        if km > 4:
            nc.sync.dma_start(x2n[:, 4:km, :], x2r_d[:, 4:km, :])
        nc.sync.dma_start(x1n[:, 4:6, :], x1r_d[:, 4:6, :])
        nc.sync.dma_start(x1n[:, 6:8, :], x1r_d[:, 6:8, :])

        # ---------------- PE warmup (keeps the HAM clock busy) -------------
        def warm(n_, base):
            for i in range(n_):
                pw = psd.tile([P, P], BF16, tag="small", name=f"warm{base + i}")
                nc.tensor.transpose(pw[:], identb[:], identb[:])

        warm(10, 0)

        # ---------------- W prep ----------------
        pwc = psd.tile([P, 16], F32, tag="small", name="pwc")
        for c in range(12):
            nc.tensor.transpose(pwc[:, c:c + 1], wrow[0:1, c * P:(c + 1) * P],
                                ident[0:1, 0:1])
        wcols = big.tile([P, 12], F32)
        nc.vector.tensor_copy(wcols[:], pwc[:, 0:12])
        w3rec = big.tile([P, 4], F32)
        nc.vector.reciprocal(w3rec[:], wcols[:, 8:12])
        u1f = big.tile([P, 4], F32)
        nc.vector.tensor_mul(u1f[:], wcols[:, 0:4], w3rec[:])
        u1r = big.tile([P, 4], BF16)       # w1/(w3*SX): recovers s1 from x1w3T
        nc.vector.tensor_scalar_mul(u1r[:], u1f[:], 1.0 / SX)
        w2r = big.tile([P, 4], BF16)
        nc.vector.tensor_copy(w2r[:], wcols[:, 4:8])
        # W3bc = broadcast row of w3*SX (outer product), for x1s = x1 * w3 * SX
        psW = psd.tile([P, D], F32, tag="small", name="psW")
        nc.tensor.matmul(psW[:], onesrow[0:1, :], wrow[0:1, 2 * D:3 * D],
                         start=True, stop=True)
        W3bc = big.tile([P, D], F32)
        nc.vector.tensor_scalar_mul(W3bc[:], psW[:], SX)

        logm1 = rows.tile([1, N], F32)
        nc.vector.tensor_scalar_mul(logm1[:], m1row[:], NEGB)
        logm2 = rows.tile([1, M], F32)
        nc.vector.tensor_scalar_mul(logm2[:], m2row[:], NEGB)

        warm(10, 10)

        # ---------------- conversions + xbar transposes ----------------
        x2b = big.tile([P, km, D], BF16)
        x1s = big.tile([P, NT, D], BF16)
        x1w3T = big.tile([P, NT, DC, P], BF16)   # (d_lo, t, c, n_lo)
        x2T = big.tile([P, km, DC, P], BF16)     # (d_lo, u, c, m_lo)

        # x2 bf16 on DVE (fast CAST), x1*w3 on gpsimd, in load-arrival order
        for u0, uw in _chunks(min(4, km), 2):
            nc.vector.tensor_copy(x2b[:, u0:u0 + uw, :], x2n[:, u0:u0 + uw, :])
        for t in range(4):
            nc.gpsimd.tensor_mul(x1s[:, t, :], x1n[:, t, :], W3bc[:])
        if km > 4:
            nc.vector.tensor_copy(x2b[:, 4:km, :], x2n[:, 4:km, :])
        for t in range(4, NT):
            nc.gpsimd.tensor_mul(x1s[:, t, :], x1n[:, t, :], W3bc[:])

        # all DMA-xbar transposes ride the SP HWDGE ring (triggers are ~1.3us
        # each on the issuing engine; SP is otherwise idle)
        nc.sync.dma_start_transpose(x2T[:, 0:min(4, km), :, :],
                                    x2b[:, 0:min(4, km), :])
        nc.sync.dma_start_transpose(x1w3T[:, 0:2, :, :], x1s[:, 0:2, :])
        nc.sync.dma_start_transpose(x1w3T[:, 2:4, :, :], x1s[:, 2:4, :])
        if km > 4:
            nc.sync.dma_start_transpose(x2T[:, 4:km, :, :], x2b[:, 4:km, :])

        # ---------------- sim + s1/s2 + E ----------------
        E_raw = big.tile([P, NT, vm], BF16)      # exp(sim), n-major
        ETraw = big.tile([P, NT, km, P], BF16)   # (m_lo, t, u, n_lo)
        ET8 = big.tile([P, NT, km, P], F8)
        x1aug = big.tile([P, kn, D], BF16)       # x1 * g1
        x2aug = big.tile([P, km, D], F8)         # x2 * g2/4
        mch = _chunks(vm)

        def sim_tile(t):
            ps = psb.tile([P, 1024], F32, tag="ps_sim", name=f"sim{t}")
            for off, w in mch:
                u0, nu = off // P, w // P
                for c in range(DC):
                    nc.tensor.matmul(ps[:, off:off + w],
                                     x1w3T[:, t, c, :],
                                     x2T[:, u0:u0 + nu, c, :],
                                     start=(c == 0), stop=(c == DC - 1))
            # one wide eviction (Act); exp undoes the SX prescale
            nc.scalar.activation(E_raw[:, t, :], ps[:, 0:vm], EXP, scale=1.0 / SX)
            # E transposes ride the SP ring (loads are done by the time they go)
            nc.sync.dma_start_transpose(ETraw[:, t, :, :], E_raw[:, t, :])

        def s_chunk(name, lhs, rhsT, brow, logm, off, w):
            t0, ntile = off // P, w // P
            ps_s = psd.tile([1, D], F32, tag="small", name=f"ps{name}{off}")
            for c in range(DC):
                nc.tensor.matmul(ps_s[0:1, 0:w], lhs[:, c:c + 1],
                                 rhsT[:, t0:t0 + ntile, c, :],
                                 start=(c == 0), stop=(c == DC - 1))
            nc.vector.tensor_add(brow[:, off:off + w], ps_s[0:1, 0:w],
                                 logm[:, off:off + w])

        def col_of(name, brow, nt):
            pbc = psd.tile([P, 16], F32, tag="small", name=f"pbc{name}")
            for t in range(nt):
                nc.tensor.transpose(pbc[:, t:t + 1], brow[0:1, t * P:(t + 1) * P],
                                    ident[0:1, 0:1])
            return pbc

        b1row = rows.tile([1, N], F32)
        b2row = rows.tile([1, M], F32)

        # s2 first (x2T is complete before any x1w3T pair), then s1 chunk A
        for off, w in _chunks(vm):
            s_chunk("b2", w2r, x2T, b2row, logm2, off, w)
        pbc2 = col_of("b2", b2row, km)
        g2c4 = big.tile([P, km], F32)
        nc.scalar.activation(g2c4[:], pbc2[:, 0:km], EXP, bias=negln4[:, 0:1])
        g2c8 = big.tile([P, 8, 16], F8)
        for u in range(km):
            nc.vector.tensor_copy(g2c8[:, u, 0:1], g2c4[:, u:u + 1])
        s_chunk("b1", u1r, x1w3T, b1row, logm1, 0, 512)

        # x2aug on Act, interleaved with E evictions (ready long before U_row)
        nc.sync.dma_start_transpose(x1w3T[:, 4:6, :, :], x1s[:, 4:6, :])
        nc.sync.dma_start_transpose(x1w3T[:, 6:8, :, :], x1s[:, 6:8, :])
        sim_tile(0)
        for u in range(km):
            nc.scalar.activation(x2aug[:, u, :], x2n[:, u, :], COPY,
                                 scale=g2c4[:, u:u + 1])
        sim_tile(1)
        sim_tile(2)
        sim_tile(3)
        s_chunk("b1", u1r, x1w3T, b1row, logm1, 512, 512)
        pbc1 = col_of("b1", b1row, NT)
        g1c = big.tile([P, NT], F32)
        nc.scalar.activation(g1c[:], pbc1[:, 0:NT], EXP)
        g1cb = big.tile([P, 8, 16], BF16)
        for k in range(kn):
            nc.vector.tensor_copy(g1cb[:, k, 0:1], g1c[:, k:k + 1])
        # x1aug on DVE right after g1c — U_col is gated on it
        for t in range(kn):
            nc.vector.tensor_scalar_mul(x1aug[:, t, :], x1n[:, t, :],
                                        g1c[:, t:t + 1])
        for t in range(4, NT):
            sim_tile(t)

        # ET8 = fp8(ETraw) on DVE (plain CAST is the fast DVE path)
        for t in range(NT):
            nc.vector.tensor_copy(ET8[:, t, :, :], ETraw[:, t, :, :])

        # block 0 = x1 (SP ring, queued after the E transposes)
        nc.sync.dma_start(out_r[:, 0:4, 0:D], x1n[:, 0:4, :])
        nc.sync.dma_start(out_r[:, 4:8, 0:D], x1n[:, 4:8, :])

        # ---------------- den2, U_col -> Q2C ----------------
        kp_m = _pairs(km)
        den2row = rows.tile([1, vm], F32)
        for off, w in _chunks(vm):
            ps_d = psd.tile([1, D], F32, tag="small", name=f"psden2{off}")
            for k in range(kn):
                nc.tensor.matmul(ps_d[0:1, 0:w], g1cb[:, k, 0:1],
                                 E_raw[:, k, off:off + w],
                                 start=(k == 0), stop=(k == kn - 1))
            nc.vector.tensor_copy(den2row[:, off:off + w], ps_d[0:1, 0:w])
        pdc2 = col_of("d2", den2row, km)
        rden2 = big.tile([P, km], F32)
        nc.vector.reciprocal(rden2[:], pdc2[:, 0:km])
        rQ = big.tile([P, km], F32)          # rden2 * g2/4 * SQ
        nc.vector.tensor_mul(rQ[:], rden2[:], g2c4[:])
        nc.vector.tensor_scalar_mul(rQ[:], rQ[:], SQ)

        Q2C = big.tile([P, km, D], F8)       # q2c * g2/4 * SQ
        for u in range(km):
            pu = psu.tile([P, D], F32, tag="ps_uv", name=f"pu{u}")
            for k in range(kn):
                nc.tensor.matmul(pu[:], E_raw[:, k, u * P:(u + 1) * P],
                                 x1aug[:, k, :], start=(k == 0),
                                 stop=(k == kn - 1))
            nc.scalar.activation(Q2C[:, u, :], pu[:], COPY, scale=rQ[:, u:u + 1])

        # ---------------- den1 ----------------
        den1row = rows.tile([1, N], F32)
        for t in range(NT):
            ps_d = psd.tile([1, D], F32, tag="small", name=f"psden1{t}")
            for i, (k0, pair) in enumerate(kp_m):
                last = i == len(kp_m) - 1
                if pair:
                    nc.tensor.matmul(ps_d[0:1, 0:P], g2c8[:, k0:k0 + 2, 0:1],
                                     ET8[:, t, k0:k0 + 2, :],
                                     start=(i == 0), stop=last, perf_mode=DR)
                else:
                    nc.tensor.matmul(ps_d[0:1, 0:P], g2c8[:, k0, 0:1],
                                     ET8[:, t, k0, :], start=(i == 0), stop=last)
            nc.vector.tensor_copy(den1row[:, t * P:(t + 1) * P], ps_d[0:1, 0:P])
        pdc1 = col_of("d1", den1row, NT)
        rden1 = big.tile([P, NT], F32)
        nc.vector.reciprocal(rden1[:], pdc1[:, 0:NT])
        rden1q = big.tile([P, NT], F32)
        nc.vector.tensor_scalar_mul(rden1q[:], rden1[:], 1.0 / SQ)
        x1rq = big.tile([P, NT, D], F32)     # x1 * rden1/SQ (pre, off hot path)
        for t in range(NT):
            nc.vector.tensor_scalar_mul(x1rq[:, t, :], x1n[:, t, :],
                                        rden1q[:, t:t + 1])

        # ---------------- U_row -> c2q ; out blocks 1, 2 ----------------
        def uv_mm(ps_ap, t, rhs):
            for i, (k0, pair) in enumerate(kp_m):
                last = i == len(kp_m) - 1
                if pair:
                    nc.tensor.matmul(ps_ap, ET8[:, t, k0:k0 + 2, :],
                                     rhs[:, k0:k0 + 2, :],
                                     start=(i == 0), stop=last, perf_mode=DR)
                else:
                    nc.tensor.matmul(ps_ap, ET8[:, t, k0, :], rhs[:, k0, :],
                                     start=(i == 0), stop=last)

        for t in range(NT):
            pr = psu.tile([P, D], F32, tag="ps_uv", name=f"pr{t}")
            uv_mm(pr[:], t, x2aug)
            combo = work.tile([P, 2 * D], F32, tag="ev", name=f"cb{t}")
            nc.scalar.activation(combo[:, 0:D], pr[:], COPY,
                                 scale=rden1[:, t:t + 1])
            nc.gpsimd.tensor_mul(combo[:, D:2 * D], x1n[:, t, :], combo[:, 0:D])
            nc.sync.dma_start(out_r[:, t, D:3 * D], combo[:])

        # ---------------- V -> q2c_att ; out block 3 ----------------
        for t in range(NT):
            pv = psu.tile([P, D], F32, tag="ps_uv", name=f"pv{t}")
            uv_mm(pv[:], t, Q2C)
            prod = work.tile([P, D], F32, tag="pd", name=f"pd{t}")
            nc.vector.tensor_mul(prod[:], x1rq[:, t, :], pv[:])
            nc.sync.dma_start(out_r[:, t, 3 * D:4 * D], prod[:])

    nc.compile()
    return nc


def _kept_tiles(mask):
    """Tiles (of 128) up to and including the last one with any valid row."""
    valid = ~mask.astype(bool)
    any_valid = valid.reshape(valid.shape[0], -1, P).any(axis=2).any(axis=0)
    nz = np.nonzero(any_valid)[0]
    return int(nz[-1]) + 1 if len(nz) else 1


def _get_nc(kn, km):
    key = (kn, km)
    if key not in _CACHE:
        _CACHE[key] = _build(kn, km)
    return _CACHE[key]


def _run(inputs, trace=False, trace_cores=None):
    x1 = np.ascontiguousarray(np.asarray(inputs["x1"], dtype=np.float32))
    x2 = np.ascontiguousarray(np.asarray(inputs["x2"], dtype=np.float32))
    m1 = np.ascontiguousarray(np.asarray(inputs["x1_mask"]).astype(np.uint8))
    m2 = np.ascontiguousarray(np.asarray(inputs["x2_mask"]).astype(np.uint8))
    W = np.ascontiguousarray(np.asarray(inputs["W"], dtype=np.float32))
    nc = _get_nc(_kept_tiles(m1), _kept_tiles(m2))
    in_maps = [
        {"x1": x1[i], "x2": x2[i], "x1_mask": m1[i], "x2_mask": m2[i], "W": W}
        for i in range(N_CORES)
    ]
    res = run_bass_kernel_spmd(nc, in_maps, core_ids=list(range(N_CORES)),
                               trace=trace, trace_cores=trace_cores)
    out = np.stack([res.results[i]["out"] for i in range(N_CORES)], axis=0)
    return out.astype(np.float32), res


def kernel(x1, x1_mask, x2, x2_mask, W, bias=None, **_kw):
    # bias is mathematically irrelevant: a global additive constant cancels in
    # both softmaxes, and every output term is softmax-weighted.
    out, _ = _run({"x1": x1, "x1_mask": x1_mask, "x2": x2, "x2_mask": x2_mask,
                   "W": W})
    return out
